# revision 9
# baseline (speedup 1.0000x reference)
"""Trainium2 Bass kernel: fused affine (x @ w + b) + row softmax.

Problem: inp [4096, 64, 14, 14] f32, w [12544, 1000] f32, b [1000] f32
         out = softmax(inp.reshape(4096, -1) @ w + b, axis=-1)   [4096, 1000] f32

Sharding: data-parallel over batch across 8 NeuronCores (512 rows/core),
w and b replicated. Softmax is row-local, so no collectives.

Per-core kernel design:
  - Host pre-transposes the x shard to K-major tiles [128, 98, 512] so the
    contraction dim lands on SBUF partitions with contiguous DMA lines.
    w is laid out [128, 98, 1000] the same way.
  - All 8 PSUM banks hold the 4 (M-tile) x 2 (N-chunk 512+488) logits
    accumulators; w and x stream through SBUF once (K-chunked DMAs).
  - Bias is injected into PSUM with a K=1 matmul against a ones-vector, so
    the ScalarE exp reads PSUM directly and emits the row-sum via accum_out.
  - Matmuls run in float32r (full-rate fp32 path for moving dim >= 256).
"""

import numpy as np

import concourse.bass as bass
import concourse.mybir as mybir
import concourse.tile as tile
from concourse.bass_utils import run_bass_kernel_spmd

P = 128
B, C, H, W, D = 4096, 64, 14, 14, 1000
K = C * H * W            # 12544
NCORES = 8
M = B // NCORES          # 512 rows per core


def build(nc_k_tiles=98, kb=7, m=M, d=D, ring=3):
    """Build the per-core kernel in raw Bass with manual synchronization.

    This walrus lowers matmul (LDWEIGHTS slot) and DMA instructions to ISA
    structs with a SINGLE sync-wait slot, so Tile's auto-generated multi-wait
    instructions fail codegen ("Too many sync wait commands"). Raw bass lets
    us put every wait on its own sequencer wait_ge instruction.

    Engine plan:
      SP:  ring-buffered chunk DMAs (combined x|w layout), then output DMAs.
      PE:  bias matmuls (ones x b broadcast) + 98x8 fp32r matmuls into all
           8 PSUM banks (4 M-tiles x 2 N-chunks of 512/488).
      ACT: exp(PSUM) -> SBUF with row-sums via accum_out.
      DVE: partial-sum add, reciprocal, scale by 1/sum.
    """
    f32 = mybir.dt.float32
    f32r = mybir.dt.float32r
    kt = nc_k_tiles
    chunks = kt // kb
    assert chunks * kb == kt
    mt = m // P
    row = m + d  # combined x|w row width per k-tile
    nsplits = []
    n0 = 0
    while n0 < d:
        nsz = min(512, d - n0)
        nsplits.append((n0, nsz))
        n0 += nsz
    nbanks = mt * len(nsplits)
    assert nbanks <= 8

    nc = bass.Bass()
    xw = nc.declare_dram_parameter("xw", [P, kt, row], f32r, isOutput=False)
    cst = nc.declare_dram_parameter("cst", [1, P + d], f32r, isOutput=False)
    out = nc.declare_dram_parameter("out", [m, d], f32, isOutput=True)

    from contextlib import ExitStack

    with ExitStack() as ctx:
        ring_sb = ctx.enter_context(nc.sbuf_tensor("ring", [P, ring, kb, row], f32r))
        cst_sb = ctx.enter_context(nc.sbuf_tensor("cst_sb", [1, P + d], f32r))
        e_sb = ctx.enter_context(nc.sbuf_tensor("e_sb", [P, mt, d], f32))
        parts_sb = ctx.enter_context(
            nc.sbuf_tensor("parts", [P, mt, len(nsplits)], f32)
        )
        tot_sb = ctx.enter_context(nc.sbuf_tensor("tot", [P, mt, 1], f32))
        rec_sb = ctx.enter_context(nc.sbuf_tensor("rec", [P, mt, 1], f32))
        ps = ctx.enter_context(nc.psum_tensor("ps", [P, nbanks, 512], f32))
        # One semaphore per concurrent-DMA stream: a sem with at most one
        # in-flight incrementer never races (DMA completions across queues
        # are not ordered, so cumulative multi-DMA counts are unsafe).
        cst_sem = ctx.enter_context(nc.semaphore("cst_sem"))
        slot_sems = [
            ctx.enter_context(nc.semaphore(f"slot_sem{s}")) for s in range(ring)
        ]
        out_sems = [
            ctx.enter_context(nc.semaphore(f"out_sem{mi}")) for mi in range(mt)
        ]
        pe_sem = ctx.enter_context(nc.semaphore("pe_sem"))
        act_sem = ctx.enter_context(nc.semaphore("act_sem"))
        dve_sem = ctx.enter_context(nc.semaphore("dve_sem"))
        chain_sem = ctx.enter_context(nc.semaphore("chain_sem"))
        block = ctx.enter_context(nc.Block())

        ones = cst_sb[0:1, 0:P]
        brow = cst_sb[0:1, P : P + d]

        @block.sync
        def _(sync):
            sync.dma_start(cst_sb[:], cst[:]).then_inc(cst_sem, 16)
            for c in range(chunks):
                if c >= ring:
                    # Slot reuse: wait until chunk (c - ring)'s matmuls read it.
                    sync.wait_ge(pe_sem, c - ring + 1)
                sync.dma_start(
                    ring_sb[:, c % ring], xw[:, c * kb : (c + 1) * kb, :]
                ).then_inc(slot_sems[c % ring], 16)
            for mi in range(mt):
                sync.wait_ge(dve_sem, mi + 1)
                sync.dma_start(
                    out[mi * P : (mi + 1) * P, :], e_sb[:, mi, :]
                ).then_inc(out_sems[mi], 16)
            # Ensure all output DMAs land before the program retires.
            for mi in range(mt):
                sync.wait_ge(out_sems[mi], 16)

        @block.tensor
        def _(tensor):
            tensor.wait_ge(cst_sem, 16)  # consts loaded
            for mi in range(mt):
                for j, (n0, nsz) in enumerate(nsplits):
                    nc.tensor.matmul(
                        ps[:, mi * len(nsplits) + j, :nsz],
                        lhsT=ones,
                        rhs=brow[:, n0 : n0 + nsz],
                        start=True,
                        stop=False,
                    )
            for c in range(chunks):
                # chunk c landed in its ring slot
                tensor.wait_ge(slot_sems[c % ring], 16 * (c // ring + 1))
                last_mm = None
                for s in range(kb):
                    for mi in range(mt):
                        for j, (n0, nsz) in enumerate(nsplits):
                            last = (c == chunks - 1) and (s == kb - 1)
                            last_mm = nc.tensor.matmul(
                                ps[:, mi * len(nsplits) + j, :nsz],
                                lhsT=ring_sb[:, c % ring, s, mi * P : (mi + 1) * P],
                                rhs=ring_sb[:, c % ring, s, m + n0 : m + n0 + nsz],
                                start=False,
                                stop=last,
                            )
                last_mm.then_inc(pe_sem, 1)  # MMs complete in pc order

        @block.scalar
        def _(scalar):
            scalar.wait_ge(pe_sem, chunks)  # all accumulation done
            for mi in range(mt):
                a = None
                for j, (n0, nsz) in enumerate(nsplits):
                    a = nc.scalar.activation(
                        e_sb[:, mi, n0 : n0 + nsz],
                        ps[:, mi * len(nsplits) + j, :nsz],
                        mybir.ActivationFunctionType.Exp,
                        accum_out=parts_sb[:, mi, j : j + 1],
                    )
                a.then_inc(act_sem, 1)

        @block.vector
        def _(vector):
            # DVE is deeply pipelined: consecutive same-engine ops with a
            # data dependency still need an explicit sem sync between them.
            for mi in range(mt):
                vector.wait_ge(act_sem, mi + 1)
                nc.vector.tensor_add(
                    tot_sb[:, mi, :], parts_sb[:, mi, 0:1], parts_sb[:, mi, 1:2]
                ).then_inc(chain_sem, 1)
                vector.wait_ge(chain_sem, 2 * mi + 1)
                nc.vector.reciprocal(
                    rec_sb[:, mi, :], tot_sb[:, mi, :]
                ).then_inc(chain_sem, 1)
                vector.wait_ge(chain_sem, 2 * mi + 2)
                nc.vector.tensor_scalar_mul(
                    e_sb[:, mi, :], e_sb[:, mi, :], rec_sb[:, mi, :]
                ).then_inc(dve_sem, 1)

    return nc


def build_tile(nc_k_tiles=98, kb=7, m=M, d=D, data_bufs=3):
    """Tile-scheduled variant (kept for reference; hits the walrus
    single-sync-wait limit on this container's compiler)."""
    f32 = mybir.dt.float32
    f32r = mybir.dt.float32r
    kt = nc_k_tiles
    chunks = kt // kb
    assert chunks * kb == kt
    mt = m // P
    row = m + d  # combined x|w row width per k-tile
    nsplits = []
    n0 = 0
    while n0 < d:
        nsz = min(512, d - n0)
        nsplits.append((n0, nsz))
        n0 += nsz

    nc = bass.Bass()
    xw = nc.declare_dram_parameter("xw", [P, kt, row], f32r, isOutput=False)
    cst = nc.declare_dram_parameter("cst", [1, P + d], f32r, isOutput=False)
    out = nc.declare_dram_parameter("out", [m, d], f32, isOutput=True)

    with tile.TileContext(nc) as tc:
        with (
            tc.tile_pool(name="consts", bufs=1) as consts,
            tc.tile_pool(name="data", bufs=data_bufs) as data,
            tc.tile_pool(name="psum", bufs=1, space="PSUM") as psum,
            tc.tile_pool(name="res", bufs=2) as res,
            tc.tile_pool(name="stats", bufs=2) as stats,
        ):
            cst_sb = consts.tile([1, P + d], f32r, name="cst_sb")
            nc.sync.dma_start(cst_sb[:], cst[:])
            ones = cst_sb[:, 0:P]
            brow = cst_sb[:, P : P + d]

            psums = [
                [
                    psum.tile([P, nsz], f32, name=f"ps_{mi}_{j}", tag=f"ps_{mi}_{j}")
                    for j, (_, nsz) in enumerate(nsplits)
                ]
                for mi in range(mt)
            ]

            # Bias broadcast into PSUM: psum[mi][j][p, n] = 1 * b[n0 + n].
            # Also opens the accumulation group (start=True resets the bank).
            for mi in range(mt):
                for j, (n0, nsz) in enumerate(nsplits):
                    nc.tensor.matmul(
                        psums[mi][j][:],
                        lhsT=ones,
                        rhs=brow[:, n0 : n0 + nsz],
                        start=True,
                        stop=False,
                    )

            for c in range(chunks):
                xwt = data.tile([P, kb, row], f32r, tag="xw")
                nc.sync.dma_start(xwt[:], xw[:, c * kb : (c + 1) * kb, :])
                for s in range(kb):
                    last = (c == chunks - 1) and (s == kb - 1)
                    for mi in range(mt):
                        for j, (n0, nsz) in enumerate(nsplits):
                            nc.tensor.matmul(
                                psums[mi][j][:],
                                lhsT=xwt[:, s, mi * P : (mi + 1) * P],
                                rhs=xwt[:, s, m + n0 : m + n0 + nsz],
                                start=False,
                                stop=last,
                            )

            # Row softmax: e = exp(logits) (ScalarE, PSUM -> SBUF, with row
            # sums via accum_out), then scale by 1/sum on VectorE.
            for mi in range(mt):
                e = res.tile([P, d], f32, tag="e")
                parts = stats.tile([P, len(nsplits)], f32, tag="parts")
                for j, (n0, nsz) in enumerate(nsplits):
                    nc.scalar.activation(
                        e[:, n0 : n0 + nsz],
                        psums[mi][j][:],
                        mybir.ActivationFunctionType.Exp,
                        accum_out=parts[:, j : j + 1],
                    )
                tot = stats.tile([P, 1], f32, tag="tot")
                if len(nsplits) == 2:
                    nc.vector.tensor_add(tot[:], parts[:, 0:1], parts[:, 1:2])
                else:
                    nc.vector.reduce_sum(tot[:], parts[:], axis=mybir.AxisListType.X)
                rec = stats.tile([P, 1], f32, tag="rec")
                nc.vector.reciprocal(rec[:], tot[:])
                nc.vector.tensor_scalar_mul(e[:], e[:], rec[:])
                nc.sync.dma_start(out[mi * P : (mi + 1) * P, :], e[:])

    return nc


def _shard_inputs(inp, w, b):
    """Host-side reshape/transpose into the kernel's K-major tile layouts."""
    x = np.ascontiguousarray(inp.reshape(B, K))
    kt = K // P
    wk = w.reshape(kt, P, D).transpose(1, 0, 2)            # [128, 98, 1000]
    cst = np.empty((1, P + D), np.float32)
    cst[0, :P] = 1.0
    cst[0, P:] = b
    in_maps = []
    for ci in range(NCORES):
        xs = x[ci * M : (ci + 1) * M]                      # [512, 12544]
        xw = np.empty((P, kt, M + D), np.float32)
        xw[:, :, :M] = xs.T.reshape(kt, P, M).transpose(1, 0, 2)
        xw[:, :, M:] = wk
        in_maps.append({"xw": xw, "cst": cst})
    return in_maps


def run(inp, w, b, trace=False):
    """Run on 8 NeuronCores; returns (full output [4096, 1000], exec_time_ns)."""
    in_maps = _shard_inputs(np.asarray(inp), np.asarray(w), np.asarray(b))
    nc = build()
    res = run_bass_kernel_spmd(nc, in_maps, list(range(NCORES)), trace=trace)
    out = np.concatenate([res.results[i]["out"] for i in range(NCORES)], axis=0)
    return out, res.exec_time_ns


def bench(inp, w, b, iters=20):
    """Measure per-execution device time by pipelined repeat timing.

    Mirrors bass2jax.run_bass_via_pjrt's sharded jit, but without donation
    and with inputs held on device, so repeated calls measure only NEFF
    execution (+ per-call dispatch overhead, amortized by async dispatch).
    """
    import time

    import jax
    from jax.sharding import Mesh, NamedSharding, PartitionSpec
    from jax.experimental.shard_map import shard_map

    from concourse import bass2jax

    in_maps = _shard_inputs(np.asarray(inp), np.asarray(w), np.asarray(b))
    nc = build()
    bass2jax.install_neuronx_cc_hook()

    import concourse.mybir as mybir_

    partition_name = nc.partition_id_tensor.name if nc.partition_id_tensor else None
    in_names, out_names, out_avals, zero_outs = [], [], [], []
    for alloc in nc.m.functions[0].allocations:
        if not isinstance(alloc, mybir_.MemoryLocationSet):
            continue
        name = alloc.memorylocations[0].name
        if alloc.kind == "ExternalInput":
            if name != partition_name:
                in_names.append(name)
        elif alloc.kind == "ExternalOutput":
            out_names.append(name)
            shape = tuple(alloc.tensor_shape)
            dtype = mybir_.dt.np(alloc.dtype)
            out_avals.append(jax.core.ShapedArray(shape, dtype))
            zero_outs.append(np.zeros(shape, dtype))
    n_params = len(in_names)
    all_names = in_names + out_names
    if partition_name is not None:
        all_names = all_names + [partition_name]

    def _body(*args):
        operands = list(args)
        if partition_name is not None:
            operands.append(bass2jax.partition_id_tensor())
        outs = bass2jax._bass_exec_p.bind(
            *operands,
            out_avals=tuple(out_avals),
            in_names=tuple(all_names),
            out_names=tuple(out_names),
            lowering_input_output_aliases=(),
            sim_require_finite=True,
            sim_require_nnan=True,
            nc=nc,
        )
        return tuple(outs)

    devices = jax.devices()[:NCORES]
    mesh = Mesh(np.asarray(devices), ("core",))
    spec = PartitionSpec("core")
    sharded = jax.jit(
        shard_map(
            _body,
            mesh=mesh,
            in_specs=(spec,) * (n_params + len(out_names)),
            out_specs=(spec,) * len(out_names),
            check_rep=False,
        ),
        keep_unused=True,
    )
    concat_in = [
        np.concatenate([m[name] for m in in_maps], axis=0) for name in in_names
    ]
    concat_zeros = [
        np.zeros((NCORES * z.shape[0], *z.shape[1:]), z.dtype) for z in zero_outs
    ]
    sharding = NamedSharding(mesh, spec)
    dev_args = [jax.device_put(a, sharding) for a in concat_in + concat_zeros]

    out_arrs = sharded(*dev_args)  # compile + warm up
    jax.block_until_ready(out_arrs)

    times = []
    for _ in range(3):
        t0 = time.monotonic()
        for _ in range(iters):
            out_arrs = sharded(*dev_args)
        jax.block_until_ready(out_arrs)
        times.append((time.monotonic() - t0) / iters)
    per_exec_ns = int(min(times) * 1e9)

    out = np.asarray(out_arrs[0]).reshape(NCORES, M, D).reshape(B, D)
    return out, per_exec_ns


def kernel(inp, w, b):
    out, _ = run(inp, w, b)
    return out


# revision 20
# speedup vs baseline: 17.2672x; 17.2672x over previous
"""Trainium2 Bass kernel: fused affine (x @ w + b) + row softmax.

Problem: inp [4096, 64, 14, 14] f32, w [12544, 1000] f32, b [1000] f32
         out = softmax(inp.reshape(4096, -1) @ w + b, axis=-1)   [4096, 1000] f32

Sharding: data-parallel over batch across 8 NeuronCores (512 rows/core),
w and b replicated. Softmax is row-local, so no collectives.

Per-core kernel design:
  - Host pre-transposes the x shard to K-major tiles [128, 98, 512] so the
    contraction dim lands on SBUF partitions with contiguous DMA lines.
    w is laid out [128, 98, 1000] the same way.
  - All 8 PSUM banks hold the 4 (M-tile) x 2 (N-chunk 512+488) logits
    accumulators; w and x stream through SBUF once (K-chunked DMAs).
  - Bias is injected into PSUM with a K=1 matmul against a ones-vector, so
    the ScalarE exp reads PSUM directly and emits the row-sum via accum_out.
  - Matmuls run in float32r (full-rate fp32 path for moving dim >= 256).
"""

import numpy as np

import concourse.bass as bass
import concourse.mybir as mybir
from concourse.bass_utils import run_bass_kernel_spmd

P = 128
B, C, H, W, D = 4096, 64, 14, 14, 1000
K = C * H * W            # 12544
NCORES = 8
M = B // NCORES          # 512 rows per core


def build(nc_k_tiles=98, kb=7, m=M, d=D, ring=4, reps=1, probe_half_n=False):
    """Build the per-core kernel in raw Bass with manual synchronization.

    This walrus lowers matmul (LDWEIGHTS slot) and DMA instructions to ISA
    structs with a SINGLE sync-wait slot, so Tile's auto-generated multi-wait
    instructions fail codegen ("Too many sync wait commands"). Raw bass lets
    us put every wait on its own sequencer wait_ge instruction.

    Engine plan:
      SP:  ring-buffered chunk DMAs (combined x|w layout), then output DMAs.
      PE:  bias matmuls (ones x b broadcast) + 98x8 fp32r matmuls into all
           8 PSUM banks (4 M-tiles x 2 N-chunks of 512/488).
      ACT: exp(PSUM) -> SBUF with row-sums via accum_out.
      DVE: partial-sum add, reciprocal, scale by 1/sum.
    """
    f32 = mybir.dt.float32
    f32r = mybir.dt.float32r
    kt = nc_k_tiles
    chunks = kt // kb
    assert chunks * kb == kt
    mt = m // P
    row = m + d  # combined x|w row width per k-tile
    nsplits = []
    n0 = 0
    while n0 < d:
        nsz = min(512, d - n0)
        nsplits.append((n0, nsz))
        n0 += nsz
    nbanks = mt * len(nsplits)
    assert nbanks <= 8

    nc = bass.Bass()
    xw = nc.declare_dram_parameter("xw", [P, kt, row], f32r, isOutput=False)
    cst = nc.declare_dram_parameter("cst", [1, P + d], f32r, isOutput=False)
    out = nc.declare_dram_parameter("out", [m, d], f32, isOutput=True)

    from contextlib import ExitStack

    with ExitStack() as ctx:
        ring_sb = ctx.enter_context(nc.sbuf_tensor("ring", [P, ring, kb, row], f32r))
        cst_sb = ctx.enter_context(nc.sbuf_tensor("cst_sb", [1, P + d], f32r))
        e_sb = ctx.enter_context(nc.sbuf_tensor("e_sb", [P, mt, d], f32))
        parts_sb = ctx.enter_context(
            nc.sbuf_tensor("parts", [P, mt, len(nsplits)], f32)
        )
        tot_sb = ctx.enter_context(nc.sbuf_tensor("tot", [P, mt, 1], f32))
        rec_sb = ctx.enter_context(nc.sbuf_tensor("rec", [P, mt, 1], f32))
        ps = ctx.enter_context(nc.psum_tensor("ps", [P, nbanks, 512], f32))
        # One semaphore per concurrent-DMA stream: a sem with at most one
        # in-flight incrementer never races (DMA completions across queues
        # are not ordered, so cumulative multi-DMA counts are unsafe).
        cst_sem = ctx.enter_context(nc.semaphore("cst_sem"))
        slot_sems = [
            ctx.enter_context(nc.semaphore(f"slot_sem{s}")) for s in range(ring)
        ]
        out_sems = [
            ctx.enter_context(nc.semaphore(f"out_sem{mi}")) for mi in range(mt)
        ]
        pe_sem = ctx.enter_context(nc.semaphore("pe_sem"))
        act_sem = ctx.enter_context(nc.semaphore("act_sem"))
        dve_sem = ctx.enter_context(nc.semaphore("dve_sem"))
        chain_sem = ctx.enter_context(nc.semaphore("chain_sem"))
        block = ctx.enter_context(nc.Block())

        ones = cst_sb[0:1, 0:P]
        brow = cst_sb[0:1, P : P + d]

        @block.sync
        def _(sync):
            sync.dma_start(cst_sb[:], cst[:]).then_inc(cst_sem, 16)
            for g in range(reps * chunks):
                if g >= ring:
                    # Slot reuse: wait until chunk (g - ring)'s matmuls read it.
                    sync.wait_ge(pe_sem, g - ring + 1)
                c = g % chunks
                sync.dma_start(
                    ring_sb[:, g % ring], xw[:, c * kb : (c + 1) * kb, :]
                ).then_inc(slot_sems[g % ring], 16)
        @block.gpsimd
        def _(gpsimd):
            # Output DMAs ride SWDGE on the otherwise-idle Pool engine so the
            # SP chunk-DMA stream never blocks on softmax completion.
            for r in range(reps):
                for mi in range(mt):
                    gpsimd.wait_ge(dve_sem, r * mt + mi + 1)
                    gpsimd.dma_start(
                        out[mi * P : (mi + 1) * P, :], e_sb[:, mi, :]
                    ).then_inc(out_sems[mi], 16)
            # Ensure all output DMAs land before the program retires.
            for mi in range(mt):
                gpsimd.wait_ge(out_sems[mi], 16 * reps)

        @block.tensor
        def _(tensor):
            tensor.wait_ge(cst_sem, 16)  # consts loaded
            for r in range(reps):
                if r > 0:
                    # ACT must have read all banks of rep r-1 (PSUM collision
                    # with the start=True overwrite below is fatal).
                    tensor.wait_ge(act_sem, r * mt)
                for mi in range(mt):
                    for j, (n0, nsz) in enumerate(nsplits):
                        nc.tensor.matmul(
                            ps[:, mi * len(nsplits) + j, :nsz],
                            lhsT=ones,
                            rhs=brow[:, n0 : n0 + nsz],
                            start=True,
                            stop=False,
                        )
                for c in range(chunks):
                    g = r * chunks + c  # global chunk index
                    tensor.wait_ge(slot_sems[g % ring], 16 * (g // ring + 1))
                    last_mm = None
                    for s in range(kb):
                        for mi in range(mt):
                            for j, (n0, nsz) in enumerate(nsplits):
                                if probe_half_n and j == 1:
                                    continue  # timing probe: halve PE work
                                last = (c == chunks - 1) and (s == kb - 1)
                                last_mm = nc.tensor.matmul(
                                    ps[:, mi * len(nsplits) + j, :nsz],
                                    lhsT=ring_sb[:, g % ring, s, mi * P : (mi + 1) * P],
                                    rhs=ring_sb[:, g % ring, s, m + n0 : m + n0 + nsz],
                                    start=False,
                                    stop=last,
                                )
                    last_mm.then_inc(pe_sem, 1)  # MMs complete in pc order

        @block.scalar
        def _(scalar):
            for r in range(reps):
                scalar.wait_ge(pe_sem, (r + 1) * chunks)  # accumulation done
                for mi in range(mt):
                    if r > 0:
                        # e_sb[:, mi] still being DMA'd out from rep r-1
                        scalar.wait_ge(out_sems[mi], 16 * r)
                    a = None
                    for j, (n0, nsz) in enumerate(nsplits):
                        a = nc.scalar.activation(
                            e_sb[:, mi, n0 : n0 + nsz],
                            ps[:, mi * len(nsplits) + j, :nsz],
                            mybir.ActivationFunctionType.Exp,
                            accum_out=parts_sb[:, mi, j : j + 1],
                        )
                    a.then_inc(act_sem, 1)

        @block.vector
        def _(vector):
            # DVE is deeply pipelined: consecutive same-engine ops with a
            # data dependency still need an explicit sem sync between them.
            for r in range(reps):
                for mi in range(mt):
                    k = r * mt + mi
                    vector.wait_ge(act_sem, k + 1)
                    nc.vector.tensor_add(
                        tot_sb[:, mi, :], parts_sb[:, mi, 0:1], parts_sb[:, mi, 1:2]
                    ).then_inc(chain_sem, 1)
                    vector.wait_ge(chain_sem, 2 * k + 1)
                    nc.vector.reciprocal(
                        rec_sb[:, mi, :], tot_sb[:, mi, :]
                    ).then_inc(chain_sem, 1)
                    vector.wait_ge(chain_sem, 2 * k + 2)
                    nc.vector.tensor_scalar_mul(
                        e_sb[:, mi, :], e_sb[:, mi, :], rec_sb[:, mi, :]
                    ).then_inc(dve_sem, 1)

    return nc


def _shard_inputs(inp, w, b):
    """Host-side reshape/transpose into the kernel's K-major tile layouts."""
    x = np.ascontiguousarray(inp.reshape(B, K))
    kt = K // P
    wk = w.reshape(kt, P, D).transpose(1, 0, 2)            # [128, 98, 1000]
    cst = np.empty((1, P + D), np.float32)
    cst[0, :P] = 1.0
    cst[0, P:] = b
    in_maps = []
    for ci in range(NCORES):
        xs = x[ci * M : (ci + 1) * M]                      # [512, 12544]
        xw = np.empty((P, kt, M + D), np.float32)
        xw[:, :, :M] = xs.T.reshape(kt, P, M).transpose(1, 0, 2)
        xw[:, :, M:] = wk
        in_maps.append({"xw": xw, "cst": cst})
    return in_maps


def run(inp, w, b):
    """Run on 8 NeuronCores; returns the full output [4096, 1000]."""
    in_maps = _shard_inputs(np.asarray(inp), np.asarray(w), np.asarray(b))
    nc = build()
    res = run_bass_kernel_spmd(nc, in_maps, list(range(NCORES)))
    return np.concatenate([res.results[i]["out"] for i in range(NCORES)], axis=0)


def _make_runner(nc, in_maps):
    """Jitted sharded executable for a prebuilt Bass module, inputs held on
    device (no donation, so it can be called repeatedly)."""
    import jax
    from jax.sharding import Mesh, NamedSharding, PartitionSpec
    from jax.experimental.shard_map import shard_map

    from concourse import bass2jax

    bass2jax.install_neuronx_cc_hook()

    import concourse.mybir as mybir_

    partition_name = nc.partition_id_tensor.name if nc.partition_id_tensor else None
    in_names, out_names, out_avals, zero_outs = [], [], [], []
    for alloc in nc.m.functions[0].allocations:
        if not isinstance(alloc, mybir_.MemoryLocationSet):
            continue
        name = alloc.memorylocations[0].name
        if alloc.kind == "ExternalInput":
            if name != partition_name:
                in_names.append(name)
        elif alloc.kind == "ExternalOutput":
            out_names.append(name)
            shape = tuple(alloc.tensor_shape)
            dtype = mybir_.dt.np(alloc.dtype)
            out_avals.append(jax.core.ShapedArray(shape, dtype))
            zero_outs.append(np.zeros(shape, dtype))
    n_params = len(in_names)
    all_names = in_names + out_names
    if partition_name is not None:
        all_names = all_names + [partition_name]

    def _body(*args):
        operands = list(args)
        if partition_name is not None:
            operands.append(bass2jax.partition_id_tensor())
        outs = bass2jax._bass_exec_p.bind(
            *operands,
            out_avals=tuple(out_avals),
            in_names=tuple(all_names),
            out_names=tuple(out_names),
            lowering_input_output_aliases=(),
            sim_require_finite=True,
            sim_require_nnan=True,
            nc=nc,
        )
        return tuple(outs)

    devices = jax.devices()[:NCORES]
    mesh = Mesh(np.asarray(devices), ("core",))
    spec = PartitionSpec("core")
    sharded = jax.jit(
        shard_map(
            _body,
            mesh=mesh,
            in_specs=(spec,) * (n_params + len(out_names)),
            out_specs=(spec,) * len(out_names),
            check_rep=False,
        ),
        keep_unused=True,
    )
    concat_in = [
        np.concatenate([m[name] for m in in_maps], axis=0) for name in in_names
    ]
    concat_zeros = [
        np.zeros((NCORES * z.shape[0], *z.shape[1:]), z.dtype) for z in zero_outs
    ]
    sharding = NamedSharding(mesh, spec)
    dev_args = [jax.device_put(a, sharding) for a in concat_in + concat_zeros]
    return sharded, dev_args


def _time_runner(fn, args, iters, trials=3):
    import time

    import jax

    out = fn(*args)  # compile + warm
    jax.block_until_ready(out)
    best = float("inf")
    for _ in range(trials):
        t0 = time.monotonic()
        for _ in range(iters):
            out = fn(*args)
        jax.block_until_ready(out)
        best = min(best, (time.monotonic() - t0) / iters)
    return best, out


def _min_call_us(fn, args, n=12):
    """Min single-call wall time (each call individually blocked)."""
    import time

    import jax

    out = fn(*args)
    jax.block_until_ready(out)  # compile + warm
    best = float("inf")
    for _ in range(n):
        t0 = time.monotonic()
        out = fn(*args)
        jax.block_until_ready(out)
        best = min(best, time.monotonic() - t0)
    return best * 1e6, out


def bench(inp, w, b, r_lo=80, r_hi=160):
    """Differential device-time measurement.

    The axon tunnel adds ~60-90 ms of per-call dispatch latency, so a single
    call cannot time a ~200 us kernel. Instead the whole pipeline is
    replicated `reps` times inside one NEFF and timed at two rep counts;
    the slope (t_hi - t_lo) / (r_hi - r_lo) is the per-execution device
    time with dispatch overhead cancelled.
    """
    import gc

    import jax

    in_maps = _shard_inputs(np.asarray(inp), np.asarray(w), np.asarray(b))

    fn, args = _make_runner(build(reps=1), in_maps)
    _, out_arrs = _min_call_us(fn, args, n=1)
    out = np.asarray(out_arrs[0]).reshape(NCORES, M, D).reshape(B, D)
    del fn
    gc.collect()
    jax.clear_caches()
    gc.collect()

    ts = {}
    for reps in (r_lo, r_hi):
        fn, args = _make_runner(build(reps=reps), in_maps)
        ts[reps], _ = _min_call_us(fn, args)
        print(f"[bench] reps={reps}: min call {ts[reps]:.0f} us", flush=True)
        del fn
        gc.collect()
        jax.clear_caches()
        gc.collect()
    per_exec_ns = int((ts[r_hi] - ts[r_lo]) / (r_hi - r_lo) * 1e3)
    return out, per_exec_ns


def kernel(inp, w, b):
    return run(inp, w, b)


# revision 26
# speedup vs baseline: 17.4069x; 1.0081x over previous
"""Trainium2 Bass kernel: fused affine (x @ w + b) + row softmax.

Problem: inp [4096, 64, 14, 14] f32, w [12544, 1000] f32, b [1000] f32
         out = softmax(inp.reshape(4096, -1) @ w + b, axis=-1)   [4096, 1000] f32

Sharding: data-parallel over batch across 8 NeuronCores (512 rows/core),
w and b replicated. Softmax is row-local, so no collectives.

Per-core kernel design:
  - Host pre-transposes the x shard to K-major tiles [128, 98, 512] so the
    contraction dim lands on SBUF partitions with contiguous DMA lines.
    w is laid out [128, 98, 1000] the same way.
  - All 8 PSUM banks hold the 4 (M-tile) x 2 (N-chunk 512+488) logits
    accumulators; w and x stream through SBUF once (K-chunked DMAs).
  - Bias is injected into PSUM with a K=1 matmul against a ones-vector, so
    the ScalarE exp reads PSUM directly and emits the row-sum via accum_out.
  - Matmuls run in float32r (full-rate fp32 path for moving dim >= 256).
"""

import numpy as np

import concourse.bass as bass
import concourse.mybir as mybir
from concourse.bass_utils import run_bass_kernel_spmd

P = 128
B, C, H, W, D = 4096, 64, 14, 14, 1000
K = C * H * W            # 12544
NCORES = 8
M = B // NCORES          # 512 rows per core


def build(nc_k_tiles=98, kb=7, m=M, d=D, ring=4, reps=1, probe_half_n=False):
    """Build the per-core kernel in raw Bass with manual synchronization.

    This walrus lowers matmul (LDWEIGHTS slot) and DMA instructions to ISA
    structs with a SINGLE sync-wait slot, so Tile's auto-generated multi-wait
    instructions fail codegen ("Too many sync wait commands"). Raw bass lets
    us put every wait on its own sequencer wait_ge instruction.

    Engine plan:
      SP:  ring-buffered chunk DMAs (combined x|w layout), then output DMAs.
      PE:  bias matmuls (ones x b broadcast) + 98x8 fp32r matmuls into all
           8 PSUM banks (4 M-tiles x 2 N-chunks of 512/488).
      ACT: exp(PSUM) -> SBUF with row-sums via accum_out.
      DVE: partial-sum add, reciprocal, scale by 1/sum.
    """
    f32 = mybir.dt.float32
    f32r = mybir.dt.float32r
    kt = nc_k_tiles
    chunks = kt // kb
    assert chunks * kb == kt
    mt = m // P
    row = m + d  # combined x|w row width per k-tile
    nsplits = []
    n0 = 0
    while n0 < d:
        nsz = min(512, d - n0)
        nsplits.append((n0, nsz))
        n0 += nsz
    nbanks = mt * len(nsplits)
    assert nbanks <= 8

    nc = bass.Bass()
    xw = nc.declare_dram_parameter("xw", [P, kt, row], f32r, isOutput=False)
    cst = nc.declare_dram_parameter("cst", [1, P + d], f32r, isOutput=False)
    out = nc.declare_dram_parameter("out", [m, d], f32, isOutput=True)

    from contextlib import ExitStack

    with ExitStack() as ctx:
        ring_sb = ctx.enter_context(nc.sbuf_tensor("ring", [P, ring, kb, row], f32r))
        cst_sb = ctx.enter_context(nc.sbuf_tensor("cst_sb", [1, P + d], f32r))
        e_sb = ctx.enter_context(nc.sbuf_tensor("e_sb", [P, mt, d], f32))
        parts_sb = ctx.enter_context(
            nc.sbuf_tensor("parts", [P, mt, len(nsplits)], f32)
        )
        tot_sb = ctx.enter_context(nc.sbuf_tensor("tot", [P, mt, 1], f32))
        rec_sb = ctx.enter_context(nc.sbuf_tensor("rec", [P, mt, 1], f32))
        relay_sb = ctx.enter_context(nc.sbuf_tensor("relay", [1, 1], f32))
        ps = ctx.enter_context(nc.psum_tensor("ps", [P, nbanks, 512], f32))
        # One semaphore per concurrent-DMA stream: a sem with at most one
        # in-flight incrementer never races (DMA completions across queues
        # are not ordered, so cumulative multi-DMA counts are unsafe).
        cst_sem = ctx.enter_context(nc.semaphore("cst_sem"))
        slot_sems = [
            ctx.enter_context(nc.semaphore(f"slot_sem{s}")) for s in range(ring)
        ]
        out_sems = [
            ctx.enter_context(nc.semaphore(f"out_sem{mi}")) for mi in range(mt)
        ]
        pe_sem = ctx.enter_context(nc.semaphore("pe_sem"))
        fin_sem = ctx.enter_context(nc.semaphore("fin_sem"))
        act_sem = ctx.enter_context(nc.semaphore("act_sem"))
        dve_sem = ctx.enter_context(nc.semaphore("dve_sem"))
        chain_sem = ctx.enter_context(nc.semaphore("chain_sem"))
        block = ctx.enter_context(nc.Block())

        ones = cst_sb[0:1, 0:P]
        brow = cst_sb[0:1, P : P + d]

        @block.sync
        def _(sync):
            sync.dma_start(cst_sb[:], cst[:]).then_inc(cst_sem, 16)
            for g in range(reps * chunks):
                if g >= ring:
                    # Slot reuse: wait until chunk (g - ring)'s matmuls read it.
                    sync.wait_ge(pe_sem, g - ring + 1)
                c = g % chunks
                sync.dma_start(
                    ring_sb[:, g % ring], xw[:, c * kb : (c + 1) * kb, :]
                ).then_inc(slot_sems[g % ring], 16)
        @block.gpsimd
        def _(gpsimd):
            # Output DMAs ride SWDGE on the otherwise-idle Pool engine so the
            # SP chunk-DMA stream never blocks on softmax completion.
            for r in range(reps):
                for mi in range(mt):
                    gpsimd.wait_ge(dve_sem, r * mt + mi + 1)
                    gpsimd.dma_start(
                        out[mi * P : (mi + 1) * P, :], e_sb[:, mi, :]
                    ).then_inc(out_sems[mi], 16)
            # Ensure all output DMAs land before the program retires.
            for mi in range(mt):
                gpsimd.wait_ge(out_sems[mi], 16 * reps)

        @block.tensor
        def _(tensor):
            tensor.wait_ge(cst_sem, 16)  # consts loaded
            for r in range(reps):
                if r > 0:
                    # ACT must have read all banks of rep r-1 (PSUM collision
                    # with the start=True overwrite below is fatal).
                    tensor.wait_ge(act_sem, r * mt)
                for mi in range(mt):
                    for j, (n0, nsz) in enumerate(nsplits):
                        nc.tensor.matmul(
                            ps[:, mi * len(nsplits) + j, :nsz],
                            lhsT=ones,
                            rhs=brow[:, n0 : n0 + nsz],
                            start=True,
                            stop=False,
                        )
                for c in range(chunks):
                    g = r * chunks + c  # global chunk index
                    tensor.wait_ge(slot_sems[g % ring], 16 * (g // ring + 1))
                    last_mm = None
                    if c < chunks - 1:
                        for s in range(kb):
                            for mi in range(mt):
                                for j, (n0, nsz) in enumerate(nsplits):
                                    if probe_half_n and j == 1:
                                        continue  # timing probe: halve PE work
                                    last_mm = nc.tensor.matmul(
                                        ps[:, mi * len(nsplits) + j, :nsz],
                                        lhsT=ring_sb[:, g % ring, s, mi * P : (mi + 1) * P],
                                        rhs=ring_sb[:, g % ring, s, m + n0 : m + n0 + nsz],
                                        start=False,
                                        stop=False,
                                    )
                    else:
                        # Final chunk: finish one M-tile at a time so the
                        # softmax tail overlaps the remaining matmuls.
                        for mi in range(mt):
                            mi_last = None
                            for s in range(kb):
                                for j, (n0, nsz) in enumerate(nsplits):
                                    if probe_half_n and j == 1:
                                        continue
                                    mi_last = nc.tensor.matmul(
                                        ps[:, mi * len(nsplits) + j, :nsz],
                                        lhsT=ring_sb[:, g % ring, s, mi * P : (mi + 1) * P],
                                        rhs=ring_sb[:, g % ring, s, m + n0 : m + n0 + nsz],
                                        start=False,
                                        stop=(s == kb - 1),
                                    )
                            mi_last.then_inc(fin_sem, 1)
                            last_mm = mi_last
                    if c < chunks - 1:
                        last_mm.then_inc(pe_sem, 1)  # MMs complete in pc order
                    # Last chunk's pe_sem tick is relayed by the ACT stream
                    # (matmul structs only take one sync update).

        @block.scalar
        def _(scalar):
            for r in range(reps):
                for mi in range(mt):
                    # This M-tile's accumulation done (per-mi, so the softmax
                    # tail overlaps the final chunk's remaining matmuls).
                    scalar.wait_ge(fin_sem, r * mt + mi + 1)
                    if r > 0:
                        # e_sb[:, mi] still being DMA'd out from rep r-1
                        scalar.wait_ge(out_sems[mi], 16 * r)
                    a = None
                    for j, (n0, nsz) in enumerate(nsplits):
                        a = nc.scalar.activation(
                            e_sb[:, mi, n0 : n0 + nsz],
                            ps[:, mi * len(nsplits) + j, :nsz],
                            mybir.ActivationFunctionType.Exp,
                            accum_out=parts_sb[:, mi, j : j + 1],
                        )
                    a.then_inc(act_sem, 1)
                # Relay the final chunk's "slot readers done" tick to pe_sem
                # on a throwaway ACT op (one sync update per instruction).
                zero = nc.const_aps.tensor(0.0, (1, 1), f32)
                nc.scalar.copy(relay_sb[0:1, 0:1], zero).then_inc(pe_sem, 1)

        @block.vector
        def _(vector):
            # DVE is deeply pipelined: consecutive same-engine ops with a
            # data dependency still need an explicit sem sync between them.
            for r in range(reps):
                for mi in range(mt):
                    k = r * mt + mi
                    vector.wait_ge(act_sem, k + 1)
                    nc.vector.tensor_add(
                        tot_sb[:, mi, :], parts_sb[:, mi, 0:1], parts_sb[:, mi, 1:2]
                    ).then_inc(chain_sem, 1)
                    vector.wait_ge(chain_sem, 2 * k + 1)
                    nc.vector.reciprocal(
                        rec_sb[:, mi, :], tot_sb[:, mi, :]
                    ).then_inc(chain_sem, 1)
                    vector.wait_ge(chain_sem, 2 * k + 2)
                    nc.vector.tensor_scalar_mul(
                        e_sb[:, mi, :], e_sb[:, mi, :], rec_sb[:, mi, :]
                    ).then_inc(dve_sem, 1)

    return nc


def _shard_inputs(inp, w, b):
    """Host-side reshape/transpose into the kernel's K-major tile layouts."""
    x = np.ascontiguousarray(inp.reshape(B, K))
    kt = K // P
    wk = w.reshape(kt, P, D).transpose(1, 0, 2)            # [128, 98, 1000]
    cst = np.empty((1, P + D), np.float32)
    cst[0, :P] = 1.0
    cst[0, P:] = b
    in_maps = []
    for ci in range(NCORES):
        xs = x[ci * M : (ci + 1) * M]                      # [512, 12544]
        xw = np.empty((P, kt, M + D), np.float32)
        xw[:, :, :M] = xs.T.reshape(kt, P, M).transpose(1, 0, 2)
        xw[:, :, M:] = wk
        in_maps.append({"xw": xw, "cst": cst})
    return in_maps


def run(inp, w, b):
    """Run on 8 NeuronCores; returns the full output [4096, 1000]."""
    in_maps = _shard_inputs(np.asarray(inp), np.asarray(w), np.asarray(b))
    nc = build()
    res = run_bass_kernel_spmd(nc, in_maps, list(range(NCORES)))
    return np.concatenate([res.results[i]["out"] for i in range(NCORES)], axis=0)


def _make_runner(nc, in_maps):
    """Jitted sharded executable for a prebuilt Bass module, inputs held on
    device (no donation, so it can be called repeatedly)."""
    import jax
    from jax.sharding import Mesh, NamedSharding, PartitionSpec
    from jax.experimental.shard_map import shard_map

    from concourse import bass2jax

    bass2jax.install_neuronx_cc_hook()

    import concourse.mybir as mybir_

    partition_name = nc.partition_id_tensor.name if nc.partition_id_tensor else None
    in_names, out_names, out_avals, zero_outs = [], [], [], []
    for alloc in nc.m.functions[0].allocations:
        if not isinstance(alloc, mybir_.MemoryLocationSet):
            continue
        name = alloc.memorylocations[0].name
        if alloc.kind == "ExternalInput":
            if name != partition_name:
                in_names.append(name)
        elif alloc.kind == "ExternalOutput":
            out_names.append(name)
            shape = tuple(alloc.tensor_shape)
            dtype = mybir_.dt.np(alloc.dtype)
            out_avals.append(jax.core.ShapedArray(shape, dtype))
            zero_outs.append(np.zeros(shape, dtype))
    n_params = len(in_names)
    all_names = in_names + out_names
    if partition_name is not None:
        all_names = all_names + [partition_name]

    def _body(*args):
        operands = list(args)
        if partition_name is not None:
            operands.append(bass2jax.partition_id_tensor())
        outs = bass2jax._bass_exec_p.bind(
            *operands,
            out_avals=tuple(out_avals),
            in_names=tuple(all_names),
            out_names=tuple(out_names),
            lowering_input_output_aliases=(),
            sim_require_finite=True,
            sim_require_nnan=True,
            nc=nc,
        )
        return tuple(outs)

    devices = jax.devices()[:NCORES]
    mesh = Mesh(np.asarray(devices), ("core",))
    spec = PartitionSpec("core")
    sharded = jax.jit(
        shard_map(
            _body,
            mesh=mesh,
            in_specs=(spec,) * (n_params + len(out_names)),
            out_specs=(spec,) * len(out_names),
            check_rep=False,
        ),
        keep_unused=True,
    )
    concat_in = [
        np.concatenate([m[name] for m in in_maps], axis=0) for name in in_names
    ]
    concat_zeros = [
        np.zeros((NCORES * z.shape[0], *z.shape[1:]), z.dtype) for z in zero_outs
    ]
    sharding = NamedSharding(mesh, spec)
    dev_args = [jax.device_put(a, sharding) for a in concat_in + concat_zeros]
    return sharded, dev_args


def _time_runner(fn, args, iters, trials=3):
    import time

    import jax

    out = fn(*args)  # compile + warm
    jax.block_until_ready(out)
    best = float("inf")
    for _ in range(trials):
        t0 = time.monotonic()
        for _ in range(iters):
            out = fn(*args)
        jax.block_until_ready(out)
        best = min(best, (time.monotonic() - t0) / iters)
    return best, out


def _min_call_us(fn, args, n=12):
    """Min single-call wall time (each call individually blocked)."""
    import time

    import jax

    out = fn(*args)
    jax.block_until_ready(out)  # compile + warm
    best = float("inf")
    for _ in range(n):
        t0 = time.monotonic()
        out = fn(*args)
        jax.block_until_ready(out)
        best = min(best, time.monotonic() - t0)
    return best * 1e6, out


def bench(inp, w, b, r_lo=80, r_hi=160):
    """Differential device-time measurement.

    The axon tunnel adds ~60-90 ms of per-call dispatch latency, so a single
    call cannot time a ~200 us kernel. Instead the whole pipeline is
    replicated `reps` times inside one NEFF and timed at two rep counts;
    the slope (t_hi - t_lo) / (r_hi - r_lo) is the per-execution device
    time with dispatch overhead cancelled.
    """
    import gc

    import jax

    in_maps = _shard_inputs(np.asarray(inp), np.asarray(w), np.asarray(b))

    fn, args = _make_runner(build(reps=1), in_maps)
    _, out_arrs = _min_call_us(fn, args, n=1)
    out = np.asarray(out_arrs[0]).reshape(NCORES, M, D).reshape(B, D)
    del fn
    gc.collect()
    jax.clear_caches()
    gc.collect()

    ts = {}
    for reps in (r_lo, r_hi):
        fn, args = _make_runner(build(reps=reps), in_maps)
        ts[reps], _ = _min_call_us(fn, args)
        print(f"[bench] reps={reps}: min call {ts[reps]:.0f} us", flush=True)
        del fn
        gc.collect()
        jax.clear_caches()
        gc.collect()
    per_exec_ns = int((ts[r_hi] - ts[r_lo]) / (r_hi - r_lo) * 1e3)
    return out, per_exec_ns


def kernel(inp, w, b):
    return run(inp, w, b)
